# revision 1
# baseline (speedup 1.0000x reference)
"""Trainium2 Bass kernel for HGNN-MLP (email/url/sender heterograph).

Math (dead-code-eliminated vs the full module: out_url/out_sender are unused):
  out = relu( x_email @ Wer  +  T @ Wcomb[:12] + bias_row )[*, :] @ Wc + bc
where
  Wer      = W_email @ (Wroot_ue + Wroot_se)                       [768,128]
  T[d,0:9]  = sum over ue-edges into d of [x_url[src], 1]           (9 cols)
  T[d,9:11] = sum over se-edges into d of [x_sender[src], 1]        (2 cols)
  Wcomb    = [[W_url;b_url]@Wrel_ue ; [W_sender;b_sender]@Wrel_se]  [11,128]
  bias_row = brel_ue + brel_se + b_email @ (Wroot_ue + Wroot_se)

Distribution: 8-way data-parallel over destination emails (12500/core),
edge lists bucketed by dst partition on host; small weights replicated.
Device does: per-edge gather of 12-float augmented rows (indirect DMA,
128 edges/instruction), scatter-sum via one-hot matmuls accumulated in
PSUM per 128-email stripe, then the big x_email projection fused with the
aggregation term and classifier. No collectives.
"""
import numpy as np
from contextlib import ExitStack

import concourse.bacc as bacc
import concourse.mybir as mybir
from concourse.bass import IndirectOffsetOnAxis
from concourse.bass_utils import run_bass_kernel_spmd

F32 = mybir.dt.float32
I32 = mybir.dt.int32

N_EMAIL, N_URL, N_SENDER = 100000, 400000, 50000
NCORE = 8
EPC = 12500                  # emails per core
NSTR = 98                    # 128-email stripes (12544 >= 12500)
CPS = 25                     # chunks (of 128 edges) per stripe
NCHUNK = NSTR * CPS          # 2450
GRP = 50                     # chunks per pipeline group (2 stripes)
NGRP = NCHUNK // GRP         # 49
SLOTS = 2 * GRP              # ring slots for gather/onehot tiles
NTAB = 450001                # combined table rows (+1 zero row)
ZROW = 450000
EPAD = 12800                 # email cols padded for 25x512 blocks
NBLK, BW = 25, 512

_prog_cache = {}


def _build_program():
    if "nc" in _prog_cache:
        return _prog_cache["nc"]
    nc = bacc.Bacc("TRN2")

    tab = nc.dram_tensor("tab", (NTAB, 12), F32, kind="ExternalInput")
    src = nc.dram_tensor("src", (128, NCHUNK), I32, kind="ExternalInput")
    rel = nc.dram_tensor("rel", (128, NCHUNK), F32, kind="ExternalInput")
    xT = nc.dram_tensor("xT", (768, EPAD), F32, kind="ExternalInput")
    wer = nc.dram_tensor("wer", (768, 128), F32, kind="ExternalInput")
    wcomb = nc.dram_tensor("wcomb", (13, 128), F32, kind="ExternalInput")
    wc = nc.dram_tensor("wc", (128, 2), F32, kind="ExternalInput")
    tabinit = nc.dram_tensor("tabinit", (13, EPAD), F32, kind="ExternalInput")
    out = nc.dram_tensor("out", (2, EPAD), F32, kind="ExternalOutput")

    with ExitStack() as ctx:
        E = ctx.enter_context
        src_sb = E(nc.sbuf_tensor("src_sb", (128, NCHUNK), I32))
        rel_sb = E(nc.sbuf_tensor("rel_sb", (128, NCHUNK), F32))
        iota_sb = E(nc.sbuf_tensor("iota_sb", (128, 128), F32))
        g_sb = E(nc.sbuf_tensor("g_sb", (128, SLOTS * 12), F32))
        oh_sb = E(nc.sbuf_tensor("oh_sb", (128, SLOTS * 128), F32))
        tab_sb = E(nc.sbuf_tensor("tab_sb", (13, EPAD), F32))
        w_sb = E(nc.sbuf_tensor("w_sb", (128, 6 * 128), F32))
        wcomb_sb = E(nc.sbuf_tensor("wcomb_sb", (13, 128), F32))
        wc_sb = E(nc.sbuf_tensor("wc_sb", (128, 2), F32))
        x_sb = E(nc.sbuf_tensor("x_sb", (128, 2 * 6 * BW), F32))
        zr_sb = E(nc.sbuf_tensor("zr_sb", (128, 2 * BW), F32))
        out_sb = E(nc.sbuf_tensor("out_sb", (2, EPAD), F32))

        ps_sc = [E(nc.psum_tensor(f"ps_sc{i}", (12, 128), F32)) for i in range(4)]
        ps_z = [E(nc.psum_tensor(f"ps_z{i}", (128, BW), F32)) for i in range(2)]
        ps_o = [E(nc.psum_tensor(f"ps_o{i}", (2, BW), F32)) for i in range(2)]

        with (
            nc.Block() as block,
            nc.semaphore("ldsem") as ldsem,    # src/rel loads (16 each)
            nc.semaphore("wsem") as wsem,      # weight loads (16 each)
            nc.semaphore("isem") as isem,      # iota + tab memset done
            nc.semaphore("gsemA") as gsemA,    # gather DMAs, even groups
            nc.semaphore("gsemB") as gsemB,    # gather DMAs, odd groups
            nc.semaphore("xsemB") as xsemB,    # x blocks, odd
            nc.semaphore("dvesem") as dvesem,  # onehot groups built
            nc.semaphore("s2sem") as s2sem,    # PE finished stripe accum
            nc.semaphore("addsem") as addsem,  # DVE added stripe to table
            nc.semaphore("xsem") as xsem,      # x blocks, even
            nc.semaphore("zsem") as zsem,      # PE finished z block
            nc.semaphore("rsem") as rsem,      # relu done per block
            nc.semaphore("osem") as osem,      # classifier matmul done
            nc.semaphore("ocop") as ocop,      # out copy done
            nc.semaphore("odma") as odma,      # final store
        ):

            @block.sync
            def _(sy):
                sy.dma_start(out=src_sb[:], in_=src[:]).then_inc(ldsem, 16)
                sy.dma_start(out=rel_sb[:], in_=rel[:]).then_inc(ldsem, 16)
                for k in range(6):
                    sy.dma_start(
                        out=w_sb[:, k * 128:(k + 1) * 128],
                        in_=wer[k * 128:(k + 1) * 128, :],
                    ).then_inc(wsem, 16)
                sy.dma_start(out=wcomb_sb[:], in_=wcomb[:]).then_inc(wsem, 16)
                sy.dma_start(out=wc_sb[:], in_=wc[:]).then_inc(wsem, 16)
                sy.dma_start(out=tab_sb[:], in_=tabinit[:]).then_inc(isem, 16)
                # email feature blocks, ring of 2, runs during scatter phase
                for b in range(NBLK):
                    if b >= 2:
                        sy.wait_ge(zsem, b - 1)
                    for k in range(6):
                        sy.dma_start(
                            out=x_sb[:, (b % 2) * 6 * BW + k * BW:
                                     (b % 2) * 6 * BW + (k + 1) * BW],
                            in_=xT[k * 128:(k + 1) * 128, b * BW:(b + 1) * BW],
                        ).then_inc(xsem if b % 2 == 0 else xsemB, 16)
                sy.wait_ge(ocop, NBLK)
                sy.dma_start(out=out[:], in_=out_sb[:]).then_inc(odma, 16)
                sy.wait_ge(odma, 16)

            @block.gpsimd
            def _(gp):
                gp.iota(
                    iota_sb[:], [[1, 128]], channel_multiplier=0,
                    allow_small_or_imprecise_dtypes=True,
                ).then_inc(isem, 1)
                gp.wait_ge(ldsem, 32)
                for g in range(NGRP):
                    if g >= 2:
                        gp.wait_ge(s2sem, 2 * (g - 1))
                    for k in range(GRP):
                        j = g * GRP + k
                        sl = j % SLOTS
                        gp.indirect_dma_start(
                            out=g_sb[:, sl * 12:(sl + 1) * 12],
                            out_offset=None,
                            in_=tab[:],
                            in_offset=IndirectOffsetOnAxis(
                                ap=src_sb[:, j:j + 1], axis=0
                            ),
                        ).then_inc(gsemA if g % 2 == 0 else gsemB, 16)


            def _ve_email_block(ve, b):
                ve.wait_ge(zsem, b + 1)
                if b >= 2:
                    ve.wait_ge(osem, b - 1)
                ve.tensor_scalar_max(
                    zr_sb[:, (b % 2) * BW:(b % 2 + 1) * BW],
                    ps_z[b % 2][:],
                    0.0,
                ).then_inc(rsem, 1)
                ve.wait_ge(osem, b + 1)
                ve.tensor_copy(
                    out=out_sb[:, b * BW:(b + 1) * BW],
                    in_=ps_o[b % 2][:],
                ).then_inc(ocop, 1)

            @block.vector
            def _(ve):
                ve.wait_ge(ldsem, 32)
                ve.wait_ge(isem, 17)
                for g in range(NGRP):
                    if g >= 2:
                        ve.wait_ge(s2sem, 2 * (g - 1))
                    for k in range(GRP):
                        j = g * GRP + k
                        sl = j % SLOTS
                        inst = ve.tensor_tensor(
                            out=oh_sb[:, sl * 128:(sl + 1) * 128],
                            in0=rel_sb[:, j:j + 1].to_broadcast([128, 128]),
                            in1=iota_sb[:],
                            op=mybir.AluOpType.is_equal,
                        )
                        if k == GRP - 1:
                            inst.then_inc(dvesem, 1)
                    # adds for the two stripes of the previous group
                    if g >= 1:
                        for t in range(2):
                            s = 2 * (g - 1) + t
                            ve.wait_ge(s2sem, s + 1)
                            ve.tensor_add(
                                tab_sb[0:12, s * 128:(s + 1) * 128],
                                tab_sb[0:12, s * 128:(s + 1) * 128],
                                ps_sc[s % 4][:],
                            ).then_inc(addsem, 1)
                    if g >= 3 and (g - 3) % 2 == 0:
                        _ve_email_block(ve, (g - 3) // 2)
                for t in range(2):
                    s = 2 * (NGRP - 1) + t
                    ve.wait_ge(s2sem, s + 1)
                    ve.tensor_add(
                        tab_sb[0:12, s * 128:(s + 1) * 128],
                        tab_sb[0:12, s * 128:(s + 1) * 128],
                        ps_sc[s % 4][:],
                    ).then_inc(addsem, 1)
                # remaining email blocks
                for b in range(23, NBLK):
                    _ve_email_block(ve, b)


            def _pe_email_block(te, b):
                te.wait_ge(addsem, min(4 * (b + 1), NSTR))
                te.wait_ge(xsem if b % 2 == 0 else xsemB,
                           16 * 6 * (b // 2 + 1))
                if b >= 2:
                    te.wait_ge(rsem, b - 1)
                for k in range(6):
                    te.matmul(
                        ps_z[b % 2][:],
                        w_sb[:, k * 128:(k + 1) * 128],
                        x_sb[:, (b % 2) * 6 * BW + k * BW:
                             (b % 2) * 6 * BW + (k + 1) * BW],
                        start=(k == 0),
                        stop=False,
                    )
                te.matmul(
                    ps_z[b % 2][:],
                    wcomb_sb[:],
                    tab_sb[:, b * BW:(b + 1) * BW],
                    start=False,
                    stop=True,
                ).then_inc(zsem, 1)
                te.wait_ge(rsem, b + 1)
                if b >= 2:
                    te.wait_ge(ocop, b - 1)
                te.matmul(
                    ps_o[b % 2][:],
                    wc_sb[:],
                    zr_sb[:, (b % 2) * BW:(b % 2 + 1) * BW],
                    start=True,
                    stop=True,
                ).then_inc(osem, 1)

            @block.tensor
            def _(te):
                te.wait_ge(wsem, 16 * 8)
                for g in range(NGRP):
                    te.wait_ge(dvesem, g + 1)
                    te.wait_ge(gsemA if g % 2 == 0 else gsemB,
                               16 * GRP * (g // 2 + 1))
                    for t in range(2):
                        s = 2 * g + t
                        if s >= 4:
                            te.wait_ge(addsem, s - 3)
                        for k25 in range(CPS):
                            j = s * CPS + k25
                            sl = j % SLOTS
                            inst = te.matmul(
                                ps_sc[s % 4][:],
                                g_sb[:, sl * 12:(sl + 1) * 12],
                                oh_sb[:, sl * 128:(sl + 1) * 128],
                                start=(k25 == 0),
                                stop=(k25 == CPS - 1),
                            )
                            if k25 == CPS - 1:
                                inst.then_inc(s2sem, 1)
                    if g >= 2 and (g - 2) % 2 == 0:
                        _pe_email_block(te, (g - 2) // 2)
                # remaining email blocks
                for b in range(24, NBLK):
                    _pe_email_block(te, b)

    nc.compile()
    _prog_cache["nc"] = nc
    return nc


def _host_prep(inputs):
    f32 = np.float32
    x_email = np.asarray(inputs["x_email"], f32)
    x_url = np.asarray(inputs["x_url"], f32)
    x_sender = np.asarray(inputs["x_sender"], f32)

    # combined augmented table
    tab = np.zeros((NTAB, 12), f32)
    tab[:N_URL, 0:8] = x_url
    tab[:N_URL, 8] = 1.0
    tab[N_URL:N_URL + N_SENDER, 9] = x_sender[:, 0]
    tab[N_URL:N_URL + N_SENDER, 10] = 1.0

    # folded weights
    wroot = inputs["Wroot_ue"] + inputs["Wroot_se"]
    wer = np.ascontiguousarray((inputs["W_email"] @ wroot).astype(f32))
    wcomb = np.zeros((13, 128), f32)
    wcomb[0:8] = inputs["W_url"] @ inputs["Wrel_ue"]
    wcomb[8] = inputs["b_url"] @ inputs["Wrel_ue"]
    wcomb[9] = inputs["W_sender"][0] @ inputs["Wrel_se"]
    wcomb[10] = inputs["b_sender"] @ inputs["Wrel_se"]
    wcomb[12] = (inputs["brel_ue"] + inputs["brel_se"]
                 + inputs["b_email"] @ wroot)
    wc = np.ascontiguousarray(inputs["Wc"].astype(f32))

    # per-core edge buckets: chunk layout [slot(128 part), chunk]
    src_all = np.concatenate([
        np.asarray(inputs["src_ue"], np.int64),
        np.asarray(inputs["src_se"], np.int64) + N_URL,
    ]).astype(np.int32)
    dst_all = np.concatenate([
        np.asarray(inputs["dst_ue"], np.int32),
        np.asarray(inputs["dst_se"], np.int32),
    ])
    core_of = dst_all // EPC

    in_maps = []
    for c in range(NCORE):
        m = core_of == c
        s = src_all[m]
        d = dst_all[m] - c * EPC
        o = np.argsort(d, kind="stable")
        s, d = s[o], d[o]
        bounds = np.searchsorted(d, np.arange(NSTR + 1) * 128)
        SRC = np.full((NCHUNK, 128), ZROW, np.int32)
        REL = np.full((NCHUNK, 128), -1.0, f32)
        for st in range(NSTR):
            a, b = int(bounds[st]), int(bounds[st + 1])
            n = b - a
            assert n <= CPS * 128, f"stripe overflow core {c} stripe {st}: {n}"
            SRC[st * CPS:(st + 1) * CPS].reshape(-1)[:n] = s[a:b]
            REL[st * CPS:(st + 1) * CPS].reshape(-1)[:n] = (
                d[a:b] - st * 128).astype(f32)
        xTc = np.zeros((768, EPAD), f32)
        xTc[:, :EPC] = x_email[c * EPC:(c + 1) * EPC].T
        tabinit_np = np.zeros((13, EPAD), f32)
        tabinit_np[12] = 1.0
        in_maps.append({
            "tab": tab,
            "tabinit": tabinit_np,
            "src": np.ascontiguousarray(SRC.T),
            "rel": np.ascontiguousarray(REL.T),
            "xT": xTc,
            "wer": wer,
            "wcomb": wcomb,
            "wc": wc,
        })
    return in_maps


def kernel(**inputs):
    nc = _build_program()
    in_maps = _host_prep(inputs)
    res = None
    last_exc = None
    for _attempt in range(3):
        try:
            res = run_bass_kernel_spmd(nc, in_maps, list(range(NCORE)))
            break
        except Exception as e:  # transient device wedge recovers on retry
            last_exc = e
            import time as _time
            _time.sleep(5.0)
    if res is None:
        raise last_exc
    out = np.empty((N_EMAIL, 2), np.float32)
    bc = np.asarray(inputs["bc"], np.float32)
    for c in range(NCORE):
        out[c * EPC:(c + 1) * EPC] = res.results[c]["out"][:, :EPC].T
    return out + bc



# revision 12
# speedup vs baseline: 8.4424x; 8.4424x over previous
"""Trainium2 Bass kernel for HGNN-MLP (email/url/sender heterograph).

Math (dead-code-eliminated vs the full module: out_url/out_sender are unused):
  out = relu( x_email @ Wer  +  T @ Wcomb[:12] + bias_row ) @ Wc + bc
where
  Wer      = W_email @ (Wroot_ue + Wroot_se)                       [768,128]
  T[d,0:9]  = sum over ue-edges into d of [x_url[src], 1]           (9 cols)
  T[d,9:11] = sum over se-edges into d of [x_sender[src], 1]        (2 cols)
  Wcomb    = [[W_url;b_url]@Wrel_ue ; [W_sender;b_sender]@Wrel_se]  [11,128]
  bias_row = brel_ue + brel_se + b_email @ (Wroot_ue + Wroot_se)

Distribution: 8-way data-parallel over destination emails. Emails are
degree-sorted on host and dealt round-robin across cores, so each 128-email
stripe holds near-equal-degree emails. Each email's edges occupy one SBUF
partition: the per-stripe indirect gather pulls 12-float augmented rows so
that partition p holds all edges of email (stripe*128+p). The segment-sum is
then a strided DVE reduce per stripe (no one-hot scatter), followed by a PE
identity-transpose into the (12, emails) table consumed by the fused
projection+classifier matmuls (bf16). No collectives.
"""
import numpy as np
from contextlib import ExitStack

import ml_dtypes
import concourse.bacc as bacc
import concourse.mybir as mybir
from concourse.bass import IndirectOffsetOnAxis
from concourse.bass_utils import run_bass_kernel_spmd

F32 = mybir.dt.float32
BF16 = mybir.dt.bfloat16
I32 = mybir.dt.int32
BF = ml_dtypes.bfloat16

N_EMAIL, N_URL, N_SENDER = 100000, 400000, 50000
NCORE = 8
EPC = 12500                  # emails per core
NSTR = 98                    # 128-email stripes (12544 >= 12500)
EPAD = 12800                 # email cols padded for 25x512 blocks
NBLK, BW = 25, 512
NTAB = 450001                # combined table rows (+1 zero row)
ZROW = 450000
GW = 100                     # max src columns per gather instruction
RGRP = 4                     # gather group ring slots
RT = 8                       # t_sb stripe ring

_prog_cache = {}


def _build_program(layout=None):
    if layout is None:
        layout = _prog_cache["layout"]
    key = ("prog", tuple(layout["m_s"]))
    if key in _prog_cache:
        return _prog_cache[key]

    m_s = layout["m_s"]                    # cols per stripe
    o_s = layout["o_s"]                    # col offset per stripe
    NCOL = int(o_s[-1])
    groups = layout["groups"]              # list of (s0, s1) stripe ranges
    NG = len(groups)
    # per-stripe -> group index, and stripes completed before group g
    grp_of = np.empty(NSTR, np.int64)
    for gi, (s0, s1) in enumerate(groups):
        grp_of[s0:s1] = gi

    nc = bacc.Bacc("TRN2")

    tab = nc.dram_tensor("tab", (NTAB, 12), F32, kind="ExternalInput")
    srcc = nc.dram_tensor("srcc", (128, NCOL), I32, kind="ExternalInput")
    xT = nc.dram_tensor("xT", (768, EPAD), BF16, kind="ExternalInput")
    wer = nc.dram_tensor("wer", (768, 128), BF16, kind="ExternalInput")
    wcomb = nc.dram_tensor("wcomb", (13, 128), BF16, kind="ExternalInput")
    wc = nc.dram_tensor("wc", (128, 2), BF16, kind="ExternalInput")
    bc = nc.dram_tensor("bc", (2, 1), F32, kind="ExternalInput")
    ident = nc.dram_tensor("ident", (128, 128), F32, kind="ExternalInput")
    tabinit = nc.dram_tensor("tabinit", (13, EPAD), BF16, kind="ExternalInput")
    out = nc.dram_tensor("out", (2, EPAD), F32, kind="ExternalOutput")

    with ExitStack() as ctx:
        E = ctx.enter_context
        src_sb = E(nc.sbuf_tensor("src_sb", (128, NCOL), I32))
        g_sb = E(nc.sbuf_tensor("g_sb", (128, RGRP * GW * 12), F32))
        t_sb = E(nc.sbuf_tensor("t_sb", (128, RT * 12), F32))
        ident_sb = E(nc.sbuf_tensor("ident_sb", (128, 128), F32))
        w_sb = E(nc.sbuf_tensor("w_sb", (128, 6 * 128), BF16))
        wcomb_sb = E(nc.sbuf_tensor("wcomb_sb", (13, 128), BF16))
        wc_sb = E(nc.sbuf_tensor("wc_sb", (128, 2), BF16))
        bc_sb = E(nc.sbuf_tensor("bc_sb", (2, 1), F32))
        tab_sb = E(nc.sbuf_tensor("tab_sb", (13, EPAD), BF16))
        x_sb = E(nc.sbuf_tensor("x_sb", (128, 2 * 6 * BW), BF16))
        zr_sb = E(nc.sbuf_tensor("zr_sb", (128, 2 * BW), BF16))
        o_sb = E(nc.sbuf_tensor("o_sb", (2, EPAD), F32))

        ps_t = [E(nc.psum_tensor(f"ps_t{i}", (12, 4 * 128), F32)) for i in range(2)]
        ps_z = [E(nc.psum_tensor(f"ps_z{i}", (128, BW), F32)) for i in range(2)]
        ps_o = [E(nc.psum_tensor(f"ps_o{i}", (2, BW), F32)) for i in range(2)]

        # quad q covers stripes 4q..min(4q+4,NSTR); 25 quads, quad q <-> block q
        NQ = 25

        def quad_cols(q):
            lo = 4 * q * 128
            hi = min((4 * q + 4) * 128, NSTR * 128)
            return lo, hi

        with (
            nc.Block() as block,
            nc.semaphore("srcsem") as srcsem,  # src index load
            nc.semaphore("wsem") as wsem,      # weights/consts/tabinit loads
            nc.semaphore("gsem0") as gsem0,    # gathers done (16/group), ring
            nc.semaphore("gsem1") as gsem1,
            nc.semaphore("gsem2") as gsem2,
            nc.semaphore("gsem3") as gsem3,
            nc.semaphore("tsem") as tsem,      # stripe reduces done (1/stripe)
            nc.semaphore("psem") as psem,      # stripe transposes done
            nc.semaphore("csem") as csem,      # quad copies into tab_sb
            nc.semaphore("xsemA") as xsemA,    # x block loads, even (16/block)
            nc.semaphore("xsemB") as xsemB,    # x block loads, odd
            nc.semaphore("zsem") as zsem,      # z matmul per block
            nc.semaphore("rsem") as rsem,      # relu per block
            nc.semaphore("osem") as osem,      # classifier matmul per block
            nc.semaphore("ocop") as ocop,      # out copy per block
            nc.semaphore("odma") as odma,      # final store
        ):

            @block.sync
            def _(sy):
                sy.dma_start(out=src_sb[:], in_=srcc[:]).then_inc(srcsem, 16)
                sy.dma_start(out=tab_sb[:], in_=tabinit[:]).then_inc(wsem, 16)
                sy.dma_start(out=ident_sb[:], in_=ident[:]).then_inc(wsem, 16)
                sy.dma_start(out=wcomb_sb[:], in_=wcomb[:]).then_inc(wsem, 16)
                sy.dma_start(out=wc_sb[:], in_=wc[:]).then_inc(wsem, 16)
                sy.dma_start(out=bc_sb[:], in_=bc[:]).then_inc(wsem, 16)
                for k in range(6):
                    sy.dma_start(
                        out=w_sb[:, k * 128:(k + 1) * 128],
                        in_=wer[k * 128:(k + 1) * 128, :],
                    ).then_inc(wsem, 16)
                sy.wait_ge(ocop, NBLK)
                sy.dma_start(out=out[:], in_=o_sb[:]).then_inc(odma, 16)
                sy.wait_ge(odma, 16)

            xTv = xT[:].rearrange("(k p) j -> p k j", p=128)

            @block.scalar
            def _(sc):
                # email feature blocks, ring of 2, one DMA per block
                for b in range(NBLK):
                    if b >= 2:
                        sc.wait_ge(zsem, b - 1)
                    sc.dma_start(
                        out=x_sb[:, (b % 2) * 6 * BW:(b % 2 + 1) * 6 * BW]
                            .rearrange("p (k j) -> p k j", k=6),
                        in_=xTv[:, :, b * BW:(b + 1) * BW],
                    ).then_inc(xsemA if b % 2 == 0 else xsemB, 16)

            @block.gpsimd
            def _(gp):
                gp.wait_ge(srcsem, 16)
                for gi, (s0, s1) in enumerate(groups):
                    if gi >= RGRP:
                        # slot free once all reduces of group gi-RGRP done
                        gp.wait_ge(tsem, groups[gi - RGRP][1])
                    c0, c1 = int(o_s[s0]), int(o_s[s1])
                    gp.indirect_dma_start(
                        out=g_sb[:, (gi % RGRP) * GW * 12:
                                 (gi % RGRP) * GW * 12 + (c1 - c0) * 12],
                        out_offset=None,
                        in_=tab[:],
                        in_offset=IndirectOffsetOnAxis(
                            ap=src_sb[:, c0:c1], axis=0
                        ),
                    ).then_inc([gsem0, gsem1, gsem2, gsem3][gi % 4], 16)

            def _ve_block(ve, b):
                # relu block b (z matmul must be done), then out copy b-1
                ve.wait_ge(zsem, b + 1)
                ve.tensor_scalar_max(
                    zr_sb[:, (b % 2) * BW:(b % 2 + 1) * BW],
                    ps_z[b % 2][:],
                    0.0,
                ).then_inc(rsem, 1)
                if b >= 1:
                    ve.wait_ge(osem, b)
                    ve.tensor_tensor(
                        out=o_sb[:, (b - 1) * BW:b * BW],
                        in0=ps_o[(b - 1) % 2][:],
                        in1=bc_sb[:].to_broadcast([2, BW]),
                        op=mybir.AluOpType.add,
                    ).then_inc(ocop, 1)

            @block.vector
            def _(ve):
                ve.wait_ge(wsem, 16 * 11)
                for s in range(NSTR):
                    gi = int(grp_of[s])
                    ve.wait_ge([gsem0, gsem1, gsem2, gsem3][gi % 4],
                               16 * (gi // 4 + 1))
                    if s >= RT:
                        ve.wait_ge(psem, s - (RT - 1))
                    s0 = groups[gi][0]
                    base = (gi % RGRP) * GW * 12 + int(o_s[s] - o_s[s0]) * 12
                    m = int(m_s[s])
                    ve.tensor_reduce(
                        out=t_sb[:, (s % RT) * 12:(s % RT + 1) * 12],
                        in_=g_sb[:, base:base + m * 12]
                            .rearrange("p (m j) -> p j m", j=12),
                        axis=mybir.AxisListType.X,
                        op=mybir.AluOpType.add,
                    ).then_inc(tsem, 1)
                    # lagged quad copy: after reduces of quad q+1, copy quad q
                    if s % 4 == 3 and s >= 7:
                        q = s // 4 - 1
                        lo, hi = quad_cols(q)
                        ve.wait_ge(psem, (hi - lo) // 128 + 4 * q)
                        ve.tensor_copy(
                            out=tab_sb[0:12, lo:hi],
                            in_=ps_t[q % 2][:, 0:hi - lo],
                        ).then_inc(csem, 1)
                        # lagged block work: relu q-1 / ocopy q-2
                        if q >= 1:
                            _ve_block(ve, q - 1)
                for q in range(NQ - 2, NQ):
                    lo, hi = quad_cols(q)
                    ve.wait_ge(psem, 4 * q + (hi - lo) // 128)
                    ve.tensor_copy(
                        out=tab_sb[0:12, lo:hi],
                        in_=ps_t[q % 2][:, 0:hi - lo],
                    ).then_inc(csem, 1)
                for b in range(NQ - 3, NQ):
                    _ve_block(ve, b)
                # final out copy for last block
                ve.wait_ge(osem, NBLK)
                ve.tensor_tensor(
                    out=o_sb[:, (NBLK - 1) * BW:NBLK * BW],
                    in0=ps_o[(NBLK - 1) % 2][:],
                    in1=bc_sb[:].to_broadcast([2, BW]),
                    op=mybir.AluOpType.add,
                ).then_inc(ocop, 1)

            def _pe_block(te, b):
                te.wait_ge(csem, b + 1)
                te.wait_ge(xsemA if b % 2 == 0 else xsemB,
                           16 * (b // 2 + 1))
                if b >= 2:
                    te.wait_ge(rsem, b - 1)
                for k in range(6):
                    te.matmul(
                        ps_z[b % 2][:],
                        w_sb[:, k * 128:(k + 1) * 128],
                        x_sb[:, (b % 2) * 6 * BW + k * BW:
                             (b % 2) * 6 * BW + (k + 1) * BW],
                        start=(k == 0),
                        stop=False,
                    )
                te.matmul(
                    ps_z[b % 2][:],
                    wcomb_sb[:],
                    tab_sb[:, b * BW:(b + 1) * BW],
                    start=False,
                    stop=True,
                ).then_inc(zsem, 1)
                te.wait_ge(rsem, b + 1)
                if b >= 2:
                    te.wait_ge(ocop, b - 1)
                te.matmul(
                    ps_o[b % 2][:],
                    wc_sb[:],
                    zr_sb[:, (b % 2) * BW:(b % 2 + 1) * BW],
                    start=True,
                    stop=True,
                ).then_inc(osem, 1)

            @block.tensor
            def _(te):
                te.wait_ge(wsem, 16 * 11)
                for s in range(NSTR):
                    te.wait_ge(tsem, s + 1)
                    q = s // 4
                    if q >= 2:
                        te.wait_ge(csem, q - 1)
                    te.matmul(
                        ps_t[q % 2][:, (s % 4) * 128:(s % 4 + 1) * 128],
                        t_sb[:, (s % RT) * 12:(s % RT + 1) * 12],
                        ident_sb[:],
                        is_transpose=True,
                    ).then_inc(psem, 1)
                    # emit block b once its quad copy can complete: copy b is
                    # emitted on DVE after reduce of stripe 4b+7
                    if s % 4 == 3 and s >= 7:
                        b = s // 4 - 1
                        if b >= 1:
                            _pe_block(te, b - 1)
                for b in range(NQ - 3, NQ):
                    _pe_block(te, b)

    nc.compile()
    _prog_cache[key] = nc
    _prog_cache["nc"] = nc
    return nc


def _host_prep(inputs):
    f32 = np.float32
    x_email = np.asarray(inputs["x_email"], f32)
    x_url = np.asarray(inputs["x_url"], f32)
    x_sender = np.asarray(inputs["x_sender"], f32)

    # combined augmented gather table
    tab = np.zeros((NTAB, 12), f32)
    tab[:N_URL, 0:8] = x_url
    tab[:N_URL, 8] = 1.0
    tab[N_URL:N_URL + N_SENDER, 9] = x_sender[:, 0]
    tab[N_URL:N_URL + N_SENDER, 10] = 1.0

    # folded weights
    wroot = inputs["Wroot_ue"] + inputs["Wroot_se"]
    wer = np.ascontiguousarray((inputs["W_email"] @ wroot).astype(f32))
    wcomb = np.zeros((13, 128), f32)
    wcomb[0:8] = inputs["W_url"] @ inputs["Wrel_ue"]
    wcomb[8] = inputs["b_url"] @ inputs["Wrel_ue"]
    wcomb[9] = inputs["W_sender"][0] @ inputs["Wrel_se"]
    wcomb[10] = inputs["b_sender"] @ inputs["Wrel_se"]
    wcomb[12] = (inputs["brel_ue"] + inputs["brel_se"]
                 + inputs["b_email"] @ wroot)

    # ---- degree-sorted layout -------------------------------------------
    dst_all = np.concatenate([
        np.asarray(inputs["dst_ue"], np.int64),
        np.asarray(inputs["dst_se"], np.int64),
    ])
    srcrow_all = np.concatenate([
        np.asarray(inputs["src_ue"], np.int64),
        np.asarray(inputs["src_se"], np.int64) + N_URL,
    ]).astype(np.int32)
    E_TOT = dst_all.shape[0]

    deg = np.bincount(dst_all, minlength=N_EMAIL)
    perm = np.argsort(-deg, kind="stable")          # emails by degree desc
    rank = np.empty(N_EMAIL, np.int64)
    rank[perm] = np.arange(N_EMAIL)

    key = rank[dst_all]
    order = np.argsort(key, kind="stable")
    ks = key[order]
    ss = srcrow_all[order]
    starts = np.searchsorted(ks, np.arange(N_EMAIL))
    k_within = np.arange(E_TOT) - starts[ks]

    core = (ks % NCORE).astype(np.int64)
    pos = ks // NCORE
    stripe = pos // 128
    part = pos % 128

    deg_rank = deg[perm]                             # descending
    dr = np.zeros(NSTR * 128 * NCORE, np.int64)
    dr[:N_EMAIL] = deg_rank                          # rank-major: pos*8+core
    m_s = dr.reshape(NSTR, 128 * NCORE).max(axis=1)
    m_s = np.maximum(m_s, 1).astype(np.int64)
    o_s = np.zeros(NSTR + 1, np.int64)
    o_s[1:] = np.cumsum(m_s)
    NCOL = int(o_s[-1])

    # gather groups: consecutive stripes, <= GW columns each
    groups = []
    s = 0
    while s < NSTR:
        s1 = s + 1
        while s1 < NSTR and o_s[s1 + 1] - o_s[s] <= GW:
            s1 += 1
        assert o_s[s1] - o_s[s] <= GW, f"stripe {s} has m={m_s[s]} > GW"
        groups.append((s, int(s1)))
        s = int(s1)

    layout = {"m_s": m_s, "o_s": o_s, "groups": groups}
    _prog_cache["layout"] = layout

    # per-core src index arrays
    SRC = np.full((NCORE, 128, NCOL), ZROW, np.int32)
    SRC[core, part, o_s[stripe] + k_within] = ss

    # constants
    identity = np.eye(128, dtype=f32)
    tabinit_np = np.zeros((13, EPAD), f32)
    tabinit_np[12] = 1.0
    tabinit_np = tabinit_np.astype(BF)
    wer_bf = wer.astype(BF)
    wcomb_bf = wcomb.astype(BF)
    wc_bf = np.ascontiguousarray(np.asarray(inputs["Wc"], f32)).astype(BF)
    bc_np = np.asarray(inputs["bc"], f32).reshape(2, 1)

    in_maps = []
    for c in range(NCORE):
        xTc = np.zeros((768, EPAD), f32)
        xTc[:, :EPC] = x_email[perm[c::NCORE]].T
        in_maps.append({
            "tab": tab,
            "srcc": np.ascontiguousarray(SRC[c]),
            "xT": xTc.astype(BF),
            "wer": wer_bf,
            "wcomb": wcomb_bf,
            "wc": wc_bf,
            "bc": bc_np,
            "ident": identity,
            "tabinit": tabinit_np,
        })
    _prog_cache["perm"] = perm
    return in_maps


def kernel(**inputs):
    in_maps = _host_prep(inputs)
    nc = _build_program(_prog_cache["layout"])
    perm = _prog_cache["perm"]
    res = None
    last_exc = None
    for _attempt in range(3):
        try:
            res = run_bass_kernel_spmd(nc, in_maps, list(range(NCORE)))
            break
        except Exception as e:  # transient device wedge recovers on retry
            last_exc = e
            import time as _time
            _time.sleep(5.0)
    if res is None:
        raise last_exc
    out = np.empty((N_EMAIL, 2), np.float32)
    for c in range(NCORE):
        out[perm[c::NCORE]] = res.results[c]["out"][:, :EPC].T
    return out


# revision 15
# speedup vs baseline: 11.2457x; 1.3321x over previous
"""Trainium2 Bass kernel for HGNN-MLP (email/url/sender heterograph).

Math (dead-code-eliminated vs the full module: out_url/out_sender are unused):
  out = relu( x_email @ Wer  +  T @ Wcomb + bias_row ) @ Wc + bc
where
  Wer      = W_email @ (Wroot_ue + Wroot_se)                       [768,128]
  T[d,0:9]  = sum over ue-edges into d of [x_url[src], 1]           (9 cols)
  T[d,9:11] = sum over se-edges into d of [x_sender[src], 1]        (2 cols)
  Wcomb    = [[W_url;b_url]@Wrel_ue ; [W_sender;b_sender]@Wrel_se]  [12,128]
  bias_row = brel_ue + brel_se + b_email @ (Wroot_ue + Wroot_se)

Distribution: 8-way data-parallel over destination emails. Emails are
degree-sorted on host and dealt round-robin across cores, so each 128-email
stripe holds near-equal-degree emails. Each email's edges occupy one SBUF
partition: the per-stripe indirect gather pulls 12-value bf16 augmented rows
so that partition p holds all edges of email (stripe*128+p). The segment-sum
is then a strided DVE reduce per stripe (no one-hot scatter), followed by a
PE identity-transpose into the (12, emails) table consumed by the fused
projection matmuls (bf16). bias_row folds into the Activation-engine relu;
bc folds into the output copy. Outputs stream per 512-email block. No
collectives.
"""
import numpy as np
from contextlib import ExitStack

import ml_dtypes
import concourse.bacc as bacc
import concourse.mybir as mybir
from concourse.bass import IndirectOffsetOnAxis
from concourse.bass_utils import run_bass_kernel_spmd

F32 = mybir.dt.float32
BF16 = mybir.dt.bfloat16
I32 = mybir.dt.int32
BF = ml_dtypes.bfloat16

N_EMAIL, N_URL, N_SENDER = 100000, 400000, 50000
NCORE = 8
EPC = 12500                  # emails per core
NSTR = 98                    # 128-email stripes (12544 >= 12500)
EPAD = 12800                 # email cols padded for 25x512 blocks
NBLK, BW = 25, 512
NTAB = 450001                # combined table rows (+1 zero row)
ZROW = 450000
GW = 100                     # max src columns per gather instruction
RGRP = 4                     # gather group ring slots
RT = 8                       # t_sb stripe ring

_prog_cache = {}


def _build_program(layout=None):
    if layout is None:
        layout = _prog_cache["layout"]
    key = ("prog", tuple(layout["m_s"]))
    if key in _prog_cache:
        return _prog_cache[key]

    m_s = layout["m_s"]                    # cols per stripe
    o_s = layout["o_s"]                    # col offset per stripe
    NCOL = int(o_s[-1])
    groups = layout["groups"]              # list of (s0, s1) stripe ranges
    grp_of = np.empty(NSTR, np.int64)
    for gi, (s0, s1) in enumerate(groups):
        grp_of[s0:s1] = gi

    nc = bacc.Bacc("TRN2")

    tab = nc.dram_tensor("tab", (NTAB, 12), BF16, kind="ExternalInput")
    srcc = nc.dram_tensor("srcc", (128, NCOL), I32, kind="ExternalInput")
    xT = nc.dram_tensor("xT", (768, EPAD), BF16, kind="ExternalInput")
    wer = nc.dram_tensor("wer", (768, 128), BF16, kind="ExternalInput")
    wcomb = nc.dram_tensor("wcomb", (12, 128), BF16, kind="ExternalInput")
    wc = nc.dram_tensor("wc", (128, 2), BF16, kind="ExternalInput")
    bc = nc.dram_tensor("bc", (2, 1), F32, kind="ExternalInput")
    zbias = nc.dram_tensor("zbias", (128, 1), F32, kind="ExternalInput")
    ident = nc.dram_tensor("ident", (128, 128), BF16, kind="ExternalInput")
    out = nc.dram_tensor("out", (2, EPAD), F32, kind="ExternalOutput")

    with ExitStack() as ctx:
        E = ctx.enter_context
        src_sb = E(nc.sbuf_tensor("src_sb", (128, NCOL), I32))
        g_sb = E(nc.sbuf_tensor("g_sb", (128, RGRP * GW * 12), BF16))
        t_sb = E(nc.sbuf_tensor("t_sb", (128, RT * 12), BF16))
        ident_sb = E(nc.sbuf_tensor("ident_sb", (128, 128), BF16))
        w_sb = E(nc.sbuf_tensor("w_sb", (128, 6 * 128), BF16))
        wcomb_sb = E(nc.sbuf_tensor("wcomb_sb", (12, 128), BF16))
        wc_sb = E(nc.sbuf_tensor("wc_sb", (128, 2), BF16))
        bc_sb = E(nc.sbuf_tensor("bc_sb", (2, 1), F32))
        zbias_sb = E(nc.sbuf_tensor("zbias_sb", (128, 1), F32))
        tab_sb = E(nc.sbuf_tensor("tab_sb", (12, EPAD), BF16))
        x_sb = E(nc.sbuf_tensor("x_sb", (128, 2 * 6 * BW), BF16))
        zr_sb = E(nc.sbuf_tensor("zr_sb", (128, 2 * BW), BF16))
        o_sb = E(nc.sbuf_tensor("o_sb", (2, EPAD), F32))

        ps_t = [E(nc.psum_tensor(f"ps_t{i}", (12, 4 * 128), BF16)) for i in range(2)]
        ps_z = [E(nc.psum_tensor(f"ps_z{i}", (128, BW), F32)) for i in range(2)]
        ps_o = [E(nc.psum_tensor(f"ps_o{i}", (2, BW), F32)) for i in range(2)]

        NQ = 25  # quad q covers stripes 4q..min(4q+4,NSTR); quad q <-> block q

        def quad_cols(q):
            lo = 4 * q * 128
            hi = min((4 * q + 4) * 128, NSTR * 128)
            return lo, hi

        with (
            nc.Block() as block,
            nc.semaphore("srcsem") as srcsem,  # src index load
            nc.semaphore("wsem") as wsem,      # weights/consts loads
            nc.semaphore("gsem0") as gsem0,    # gathers done (16/group), ring
            nc.semaphore("gsem1") as gsem1,
            nc.semaphore("gsem2") as gsem2,
            nc.semaphore("gsem3") as gsem3,
            nc.semaphore("tsem") as tsem,      # stripe reduces done (1/stripe)
            nc.semaphore("psem") as psem,      # stripe transposes done
            nc.semaphore("csem") as csem,      # quad copies into tab_sb
            nc.semaphore("xsemA") as xsemA,    # x block loads, even (16/block)
            nc.semaphore("xsemB") as xsemB,    # x block loads, odd
            nc.semaphore("zsem") as zsem,      # z matmul per block
            nc.semaphore("rsem") as rsem,      # relu per block
            nc.semaphore("osem") as osem,      # classifier matmul per block
            nc.semaphore("ocop") as ocop,      # out copy per block
            nc.semaphore("odma") as odma,      # per-block stores
        ):
            xTv = xT[:].rearrange("(k p) j -> p k j", p=128)

            def _x_load(eng, b):
                if b >= 2:
                    eng.wait_ge(zsem, b - 1)
                eng.dma_start(
                    out=x_sb[:, (b % 2) * 6 * BW:(b % 2 + 1) * 6 * BW]
                        .rearrange("p (k j) -> p k j", k=6),
                    in_=xTv[:, :, b * BW:(b + 1) * BW],
                ).then_inc(xsemA if b % 2 == 0 else xsemB, 16)

            @block.sync
            def _(sy):
                sy.dma_start(out=src_sb[:], in_=srcc[:]).then_inc(srcsem, 16)
                sy.dma_start(out=ident_sb[:], in_=ident[:]).then_inc(wsem, 16)
                sy.dma_start(out=wcomb_sb[:], in_=wcomb[:]).then_inc(wsem, 16)
                sy.dma_start(out=wc_sb[:], in_=wc[:]).then_inc(wsem, 16)
                sy.dma_start(out=bc_sb[:], in_=bc[:]).then_inc(wsem, 16)
                sy.dma_start(out=zbias_sb[:], in_=zbias[:]).then_inc(wsem, 16)
                for k in range(6):
                    sy.dma_start(
                        out=w_sb[:, k * 128:(k + 1) * 128],
                        in_=wer[k * 128:(k + 1) * 128, :],
                    ).then_inc(wsem, 16)
                # even x blocks + per-block output stores
                for b in range(NBLK):
                    if b % 2 == 0:
                        _x_load(sy, b)
                    if b >= 2:
                        sy.wait_ge(ocop, b - 1)
                        sy.dma_start(
                            out=out[:, (b - 2) * BW:(b - 1) * BW],
                            in_=o_sb[:, (b - 2) * BW:(b - 1) * BW],
                        ).then_inc(odma, 16)
                for b in (NBLK - 2, NBLK - 1):
                    sy.wait_ge(ocop, b + 1)
                    sy.dma_start(
                        out=out[:, b * BW:(b + 1) * BW],
                        in_=o_sb[:, b * BW:(b + 1) * BW],
                    ).then_inc(odma, 16)
                sy.wait_ge(odma, 16 * NBLK)

            @block.scalar
            def _(sc):
                sc.wait_ge(wsem, 16 * 11)
                # odd x blocks + relu(+bias) + output bc-add copies
                for b in range(NBLK):
                    if b % 2 == 1:
                        _x_load(sc, b)
                    # relu for block b-1
                    if b >= 1:
                        r = b - 1
                        sc.wait_ge(zsem, r + 1)
                        if r >= 2:
                            sc.wait_ge(osem, r - 1)
                        sc.activation(
                            out=zr_sb[:, (r % 2) * BW:(r % 2 + 1) * BW],
                            in_=ps_z[r % 2][:],
                            func=mybir.ActivationFunctionType.Relu,
                            bias=zbias_sb[:],
                        ).then_inc(rsem, 1)
                    # output copy (+bc) for block b-2
                    if b >= 2:
                        o = b - 2
                        sc.wait_ge(osem, o + 1)
                        sc.activation(
                            out=o_sb[:, o * BW:(o + 1) * BW],
                            in_=ps_o[o % 2][:],
                            func=mybir.ActivationFunctionType.Identity,
                            bias=bc_sb[:],
                        ).then_inc(ocop, 1)
                for r in (NBLK - 1,):
                    sc.wait_ge(zsem, r + 1)
                    sc.wait_ge(osem, r - 1)
                    sc.activation(
                        out=zr_sb[:, (r % 2) * BW:(r % 2 + 1) * BW],
                        in_=ps_z[r % 2][:],
                        func=mybir.ActivationFunctionType.Relu,
                        bias=zbias_sb[:],
                    ).then_inc(rsem, 1)
                for o in (NBLK - 2, NBLK - 1):
                    sc.wait_ge(osem, o + 1)
                    sc.activation(
                        out=o_sb[:, o * BW:(o + 1) * BW],
                        in_=ps_o[o % 2][:],
                        func=mybir.ActivationFunctionType.Identity,
                        bias=bc_sb[:],
                    ).then_inc(ocop, 1)

            @block.gpsimd
            def _(gp):
                gp.wait_ge(srcsem, 16)
                for gi, (s0, s1) in enumerate(groups):
                    if gi >= RGRP:
                        gp.wait_ge(tsem, groups[gi - RGRP][1])
                    c0, c1 = int(o_s[s0]), int(o_s[s1])
                    gp.indirect_dma_start(
                        out=g_sb[:, (gi % RGRP) * GW * 12:
                                 (gi % RGRP) * GW * 12 + (c1 - c0) * 12],
                        out_offset=None,
                        in_=tab[:],
                        in_offset=IndirectOffsetOnAxis(
                            ap=src_sb[:, c0:c1], axis=0
                        ),
                    ).then_inc([gsem0, gsem1, gsem2, gsem3][gi % 4], 16)

            @block.vector
            def _(ve):
                ve.wait_ge(wsem, 16 * 11)
                # pad cols of tab_sb (beyond stripe coverage) stay zero
                ve.memset(tab_sb[:, NSTR * 128:EPAD], 0.0)
                with nc.allow_low_precision(reason="bf16 edge aggregation"):
                    for s in range(NSTR):
                        gi = int(grp_of[s])
                        ve.wait_ge([gsem0, gsem1, gsem2, gsem3][gi % 4],
                                   16 * (gi // 4 + 1))
                        if s >= RT:
                            ve.wait_ge(psem, s - (RT - 1))
                        s0 = groups[gi][0]
                        base = (gi % RGRP) * GW * 12 + int(o_s[s] - o_s[s0]) * 12
                        m = int(m_s[s])
                        ve.tensor_reduce(
                            out=t_sb[:, (s % RT) * 12:(s % RT + 1) * 12],
                            in_=g_sb[:, base:base + m * 12]
                                .rearrange("p (m j) -> p j m", j=12),
                            axis=mybir.AxisListType.X,
                            op=mybir.AluOpType.add,
                        ).then_inc(tsem, 1)
                        # lagged quad copy: after reduces of quad q+1, copy quad q
                        if s % 4 == 3 and s >= 7:
                            q = s // 4 - 1
                            lo, hi = quad_cols(q)
                            ve.wait_ge(psem, 4 * q + (hi - lo) // 128)
                            ve.tensor_copy(
                                out=tab_sb[:, lo:hi],
                                in_=ps_t[q % 2][:, 0:hi - lo],
                            ).then_inc(csem, 1)
                for q in range(NQ - 2, NQ):
                    lo, hi = quad_cols(q)
                    ve.wait_ge(psem, 4 * q + (hi - lo) // 128)
                    ve.tensor_copy(
                        out=tab_sb[:, lo:hi],
                        in_=ps_t[q % 2][:, 0:hi - lo],
                    ).then_inc(csem, 1)

            def _pe_block(te, b):
                te.wait_ge(csem, b + 1)
                te.wait_ge(xsemA if b % 2 == 0 else xsemB,
                           16 * (b // 2 + 1))
                if b >= 2:
                    te.wait_ge(rsem, b - 1)
                for k in range(6):
                    te.matmul(
                        ps_z[b % 2][:],
                        w_sb[:, k * 128:(k + 1) * 128],
                        x_sb[:, (b % 2) * 6 * BW + k * BW:
                             (b % 2) * 6 * BW + (k + 1) * BW],
                        start=(k == 0),
                        stop=False,
                    )
                te.matmul(
                    ps_z[b % 2][:],
                    wcomb_sb[:],
                    tab_sb[:, b * BW:(b + 1) * BW],
                    start=False,
                    stop=True,
                ).then_inc(zsem, 1)
                te.wait_ge(rsem, b + 1)
                if b >= 2:
                    te.wait_ge(ocop, b - 1)
                te.matmul(
                    ps_o[b % 2][:],
                    wc_sb[:],
                    zr_sb[:, (b % 2) * BW:(b % 2 + 1) * BW],
                    start=True,
                    stop=True,
                ).then_inc(osem, 1)

            @block.tensor
            def _(te):
                te.wait_ge(wsem, 16 * 11)
                for s in range(NSTR):
                    te.wait_ge(tsem, s + 1)
                    q = s // 4
                    if q >= 2:
                        te.wait_ge(csem, q - 1)
                    te.matmul(
                        ps_t[q % 2][:, (s % 4) * 128:(s % 4 + 1) * 128],
                        t_sb[:, (s % RT) * 12:(s % RT + 1) * 12],
                        ident_sb[:],
                        is_transpose=True,
                    ).then_inc(psem, 1)
                    # emit block b once its quad copy can complete (copy b is
                    # emitted on DVE after reduce of stripe 4b+7)
                    if s % 4 == 3 and s >= 11:
                        _pe_block(te, s // 4 - 2)
                for b in range(NQ - 3, NQ):
                    _pe_block(te, b)

    nc.compile()
    _prog_cache[key] = nc
    _prog_cache["nc"] = nc
    return nc


def _host_prep(inputs):
    f32 = np.float32
    x_email = np.asarray(inputs["x_email"], f32)
    x_url = np.asarray(inputs["x_url"], f32)
    x_sender = np.asarray(inputs["x_sender"], f32)

    # combined augmented gather table
    tab = np.zeros((NTAB, 12), f32)
    tab[:N_URL, 0:8] = x_url
    tab[:N_URL, 8] = 1.0
    tab[N_URL:N_URL + N_SENDER, 9] = x_sender[:, 0]
    tab[N_URL:N_URL + N_SENDER, 10] = 1.0

    # folded weights
    wroot = inputs["Wroot_ue"] + inputs["Wroot_se"]
    wer = np.ascontiguousarray((inputs["W_email"] @ wroot).astype(f32))
    wcomb = np.zeros((12, 128), f32)
    wcomb[0:8] = inputs["W_url"] @ inputs["Wrel_ue"]
    wcomb[8] = inputs["b_url"] @ inputs["Wrel_ue"]
    wcomb[9] = inputs["W_sender"][0] @ inputs["Wrel_se"]
    wcomb[10] = inputs["b_sender"] @ inputs["Wrel_se"]
    zbias = (inputs["brel_ue"] + inputs["brel_se"]
             + inputs["b_email"] @ wroot).astype(f32).reshape(128, 1)

    # ---- degree-sorted layout -------------------------------------------
    dst_all = np.concatenate([
        np.asarray(inputs["dst_ue"], np.int64),
        np.asarray(inputs["dst_se"], np.int64),
    ])
    srcrow_all = np.concatenate([
        np.asarray(inputs["src_ue"], np.int64),
        np.asarray(inputs["src_se"], np.int64) + N_URL,
    ]).astype(np.int32)
    E_TOT = dst_all.shape[0]

    deg = np.bincount(dst_all, minlength=N_EMAIL)
    perm = np.argsort(-deg, kind="stable")          # emails by degree desc
    rank = np.empty(N_EMAIL, np.int64)
    rank[perm] = np.arange(N_EMAIL)

    key = rank[dst_all]
    order = np.argsort(key, kind="stable")
    ks = key[order]
    ss = srcrow_all[order]
    starts = np.searchsorted(ks, np.arange(N_EMAIL))
    k_within = np.arange(E_TOT) - starts[ks]

    core = (ks % NCORE).astype(np.int64)
    pos = ks // NCORE
    stripe = pos // 128
    part = pos % 128

    deg_rank = deg[perm]                             # descending
    dr = np.zeros(NSTR * 128 * NCORE, np.int64)
    dr[:N_EMAIL] = deg_rank                          # rank-major: pos*8+core
    m_s = dr.reshape(NSTR, 128 * NCORE).max(axis=1)
    m_s = np.maximum(m_s, 1).astype(np.int64)
    o_s = np.zeros(NSTR + 1, np.int64)
    o_s[1:] = np.cumsum(m_s)
    NCOL = int(o_s[-1])

    # gather groups: consecutive stripes, <= GW columns each
    groups = []
    s = 0
    while s < NSTR:
        s1 = s + 1
        while s1 < NSTR and o_s[s1 + 1] - o_s[s] <= GW:
            s1 += 1
        assert o_s[s1] - o_s[s] <= GW, f"stripe {s} has m={m_s[s]} > GW"
        groups.append((s, int(s1)))
        s = int(s1)

    layout = {"m_s": m_s, "o_s": o_s, "groups": groups}
    _prog_cache["layout"] = layout

    # per-core src index arrays
    SRC = np.full((NCORE, 128, NCOL), ZROW, np.int32)
    SRC[core, part, o_s[stripe] + k_within] = ss

    identity = np.eye(128, dtype=f32).astype(BF)
    wer_bf = wer.astype(BF)
    wcomb_bf = wcomb.astype(BF)
    wc_bf = np.ascontiguousarray(np.asarray(inputs["Wc"], f32)).astype(BF)
    bc_np = np.asarray(inputs["bc"], f32).reshape(2, 1)
    tab_bf = tab.astype(BF)

    in_maps = []
    for c in range(NCORE):
        xTc = np.zeros((768, EPAD), f32)
        xTc[:, :EPC] = x_email[perm[c::NCORE]].T
        in_maps.append({
            "tab": tab_bf,
            "srcc": np.ascontiguousarray(SRC[c]),
            "xT": xTc.astype(BF),
            "wer": wer_bf,
            "wcomb": wcomb_bf,
            "wc": wc_bf,
            "bc": bc_np,
            "zbias": zbias,
            "ident": identity,
        })
    _prog_cache["perm"] = perm
    return in_maps


def kernel(**inputs):
    in_maps = _host_prep(inputs)
    nc = _build_program(_prog_cache["layout"])
    perm = _prog_cache["perm"]
    res = None
    last_exc = None
    for _attempt in range(3):
        try:
            res = run_bass_kernel_spmd(nc, in_maps, list(range(NCORE)))
            break
        except Exception as e:  # transient device wedge recovers on retry
            last_exc = e
            import time as _time
            _time.sleep(5.0)
    if res is None:
        raise last_exc
    out = np.empty((N_EMAIL, 2), np.float32)
    for c in range(NCORE):
        out[perm[c::NCORE]] = res.results[c]["out"][:, :EPC].T
    return out


# revision 17
# speedup vs baseline: 14.2396x; 1.2662x over previous
"""Trainium2 Bass kernel for HGNN-MLP (email/url/sender heterograph).

Math (dead-code-eliminated vs the full module: out_url/out_sender are unused):
  out = relu( x_email @ Wer  +  T @ Wcomb + bias_row ) @ Wc + bc
where
  Wer      = W_email @ (Wroot_ue + Wroot_se)                       [768,128]
  T[d,0:9]  = sum over ue-edges into d of [x_url[src], 1]           (9 cols)
  T[d,9:11] = sum over se-edges into d of [x_sender[src], 1]        (2 cols)
  Wcomb    = [[W_url;b_url]@Wrel_ue ; [W_sender;b_sender]@Wrel_se]  [12,128]
  bias_row = brel_ue + brel_se + b_email @ (Wroot_ue + Wroot_se)

Distribution: 8-way data-parallel over destination emails. Emails are
degree-sorted on host and dealt round-robin across cores, so each 128-email
stripe holds near-equal-degree emails. Each email's edges occupy one SBUF
partition: the per-stripe indirect gather pulls 12-value bf16 augmented rows
so that partition p holds all edges of email (stripe*128+p). The segment-sum
is then a strided DVE reduce per stripe (no one-hot scatter), followed by a
PE identity-transpose into the (12, emails) table consumed by the fused
projection matmuls (bf16). bias_row folds into the Activation-engine relu;
bc folds into the output copy. Outputs stream per 512-email block. No
collectives.
"""
import numpy as np
from contextlib import ExitStack

import ml_dtypes
import concourse.bacc as bacc
import concourse.mybir as mybir
from concourse.bass import IndirectOffsetOnAxis
from concourse.bass_utils import run_bass_kernel_spmd

F32 = mybir.dt.float32
BF16 = mybir.dt.bfloat16
I32 = mybir.dt.int32
BF = ml_dtypes.bfloat16

N_EMAIL, N_URL, N_SENDER = 100000, 400000, 50000
NCORE = 8
EPC = 12500                  # emails per core
NSTR = 98                    # 128-email stripes (12544 >= 12500)
EPAD = 12800                 # email cols padded for 25x512 blocks
NBLK, BW = 25, 512
NTAB = 450001                # combined table rows (+1 zero row)
ZROW = 450000
GW = 100                     # max src columns per gather instruction
RGRP = 4                     # gather group ring slots
RT = 8                       # t_sb stripe ring

_prog_cache = {}


def _build_program(layout=None):
    if layout is None:
        layout = _prog_cache["layout"]
    key = ("prog", tuple(layout["m_s"]))
    if key in _prog_cache:
        return _prog_cache[key]

    m_s = layout["m_s"]                    # cols per stripe
    o_s = layout["o_s"]                    # col offset per stripe
    NCOL = int(o_s[-1])
    groups = layout["groups"]              # list of (s0, s1) stripe ranges
    grp_of = np.empty(NSTR, np.int64)
    for gi, (s0, s1) in enumerate(groups):
        grp_of[s0:s1] = gi

    nc = bacc.Bacc("TRN2")

    tab = nc.dram_tensor("tab", (NTAB, 12), BF16, kind="ExternalInput")
    srcc = nc.dram_tensor("srcc", (128, NCOL), I32, kind="ExternalInput")
    xT = nc.dram_tensor("xT", (768, EPAD), BF16, kind="ExternalInput")
    wer = nc.dram_tensor("wer", (768, 128), BF16, kind="ExternalInput")
    wcomb = nc.dram_tensor("wcomb", (12, 128), BF16, kind="ExternalInput")
    wc = nc.dram_tensor("wc", (128, 2), BF16, kind="ExternalInput")
    bc = nc.dram_tensor("bc", (2, 1), F32, kind="ExternalInput")
    zbias = nc.dram_tensor("zbias", (128, 1), F32, kind="ExternalInput")
    ident = nc.dram_tensor("ident", (128, 128), BF16, kind="ExternalInput")
    out = nc.dram_tensor("out", (2, EPAD), F32, kind="ExternalOutput")

    with ExitStack() as ctx:
        E = ctx.enter_context
        src_sb = E(nc.sbuf_tensor("src_sb", (128, NCOL), I32))
        g_sb = E(nc.sbuf_tensor("g_sb", (128, RGRP * GW * 12), BF16))
        t_sb = E(nc.sbuf_tensor("t_sb", (128, RT * 12), BF16))
        ident_sb = E(nc.sbuf_tensor("ident_sb", (128, 128), BF16))
        w_sb = E(nc.sbuf_tensor("w_sb", (128, 6 * 128), BF16))
        wcomb_sb = E(nc.sbuf_tensor("wcomb_sb", (12, 128), BF16))
        wc_sb = E(nc.sbuf_tensor("wc_sb", (128, 2), BF16))
        bc_sb = E(nc.sbuf_tensor("bc_sb", (2, 1), F32))
        zbias_sb = E(nc.sbuf_tensor("zbias_sb", (128, 1), F32))
        tab_sb = E(nc.sbuf_tensor("tab_sb", (12, EPAD), BF16))
        x_sb = E(nc.sbuf_tensor("x_sb", (128, 4 * 6 * BW), BF16))
        zr_sb = E(nc.sbuf_tensor("zr_sb", (128, 2 * BW), BF16))
        o_sb = E(nc.sbuf_tensor("o_sb", (2, EPAD), F32))

        ps_t = [E(nc.psum_tensor(f"ps_t{i}", (12, 4 * 128), BF16)) for i in range(2)]
        ps_z = [E(nc.psum_tensor(f"ps_z{i}", (128, BW), F32)) for i in range(4)]
        ps_o = [E(nc.psum_tensor(f"ps_o{i}", (2, BW), F32)) for i in range(2)]

        NQ = 25  # quad q covers stripes 4q..min(4q+4,NSTR); quad q <-> block q

        def quad_cols(q):
            lo = 4 * q * 128
            hi = min((4 * q + 4) * 128, NSTR * 128)
            return lo, hi

        with (
            nc.Block() as block,
            nc.semaphore("srcsem") as srcsem,  # src index load
            nc.semaphore("wsem") as wsem,      # weights/consts loads
            nc.semaphore("gsem0") as gsem0,    # gathers done (16/group), ring
            nc.semaphore("gsem1") as gsem1,
            nc.semaphore("gsem2") as gsem2,
            nc.semaphore("gsem3") as gsem3,
            nc.semaphore("tsem") as tsem,      # stripe reduces done (1/stripe)
            nc.semaphore("psem") as psem,      # stripe transposes done
            nc.semaphore("csem") as csem,      # quad copies into tab_sb
            nc.semaphore("xsem0") as xsem0,    # x block loads, ring of 4
            nc.semaphore("xsem1") as xsem1,
            nc.semaphore("xsem2") as xsem2,
            nc.semaphore("xsem3") as xsem3,
            nc.semaphore("zsem") as zsem,      # z matmul per block
            nc.semaphore("rsem") as rsem,      # relu per block
            nc.semaphore("osem") as osem,      # classifier matmul per block
            nc.semaphore("ocopA") as ocopA,    # out copy, even blocks (DVE)
            nc.semaphore("ocopB") as ocopB,    # out copy, odd blocks (Act)
            nc.semaphore("odma") as odma,      # per-block stores
        ):
            xTv = xT[:].rearrange("(k p) j -> p k j", p=128)

            def _x_load(eng, b):
                if b >= 4:
                    eng.wait_ge(zsem, b - 3)
                eng.dma_start(
                    out=x_sb[:, (b % 4) * 6 * BW:(b % 4 + 1) * 6 * BW]
                        .rearrange("p (k j) -> p k j", k=6),
                    in_=xTv[:, :, b * BW:(b + 1) * BW],
                ).then_inc([xsem0, xsem1, xsem2, xsem3][b % 4], 16)

            def _ocopy_act(sc, o):
                sc.wait_ge(osem, o + 1)
                sc.activation(
                    out=o_sb[:, o * BW:(o + 1) * BW],
                    in_=ps_o[o % 2][:],
                    func=mybir.ActivationFunctionType.Identity,
                    bias=bc_sb[:],
                ).then_inc(ocopB, 1)

            def _ocopy_dve(ve, o):
                ve.wait_ge(osem, o + 1)
                ve.tensor_tensor(
                    out=o_sb[:, o * BW:(o + 1) * BW],
                    in0=ps_o[o % 2][:],
                    in1=bc_sb[:].to_broadcast([2, BW]),
                    op=mybir.AluOpType.add,
                ).then_inc(ocopA, 1)

            def _relu(sc, r):
                sc.wait_ge(zsem, r + 1)
                if r >= 2:
                    sc.wait_ge(osem, r - 1)
                sc.activation(
                    out=zr_sb[:, (r % 2) * BW:(r % 2 + 1) * BW],
                    in_=ps_z[r % 4][:],
                    func=mybir.ActivationFunctionType.Relu,
                    bias=zbias_sb[:],
                ).then_inc(rsem, 1)

            @block.sync
            def _(sy):
                sy.dma_start(out=src_sb[:], in_=srcc[:]).then_inc(srcsem, 16)
                sy.dma_start(out=ident_sb[:], in_=ident[:]).then_inc(wsem, 16)
                sy.dma_start(out=wcomb_sb[:], in_=wcomb[:]).then_inc(wsem, 16)
                sy.dma_start(out=wc_sb[:], in_=wc[:]).then_inc(wsem, 16)
                sy.dma_start(out=bc_sb[:], in_=bc[:]).then_inc(wsem, 16)
                sy.dma_start(out=zbias_sb[:], in_=zbias[:]).then_inc(wsem, 16)
                for k in range(6):
                    sy.dma_start(
                        out=w_sb[:, k * 128:(k + 1) * 128],
                        in_=wer[k * 128:(k + 1) * 128, :],
                    ).then_inc(wsem, 16)
                # even x blocks
                for b in range(0, NBLK, 2):
                    _x_load(sy, b)
                sy.wait_ge(odma, 16 * NBLK)

            @block.scalar
            def _(sc):
                sc.wait_ge(wsem, 16 * 11)
                # odd x blocks + relu(+bias) + odd output copies
                for b in range(NBLK):
                    if b % 2 == 1:
                        _x_load(sc, b)
                    if b >= 1:
                        _relu(sc, b - 1)
                    if b >= 3 and b % 2 == 1:
                        _ocopy_act(sc, b - 2)
                _relu(sc, NBLK - 1)
                _ocopy_act(sc, NBLK - 2)

            @block.gpsimd
            def _(gp):
                gp.wait_ge(srcsem, 16)
                for gi, (s0, s1) in enumerate(groups):
                    if gi >= RGRP:
                        gp.wait_ge(tsem, groups[gi - RGRP][1])
                    c0, c1 = int(o_s[s0]), int(o_s[s1])
                    gp.indirect_dma_start(
                        out=g_sb[:, (gi % RGRP) * GW * 12:
                                 (gi % RGRP) * GW * 12 + (c1 - c0) * 12],
                        out_offset=None,
                        in_=tab[:],
                        in_offset=IndirectOffsetOnAxis(
                            ap=src_sb[:, c0:c1], axis=0
                        ),
                    ).then_inc([gsem0, gsem1, gsem2, gsem3][gi % 4], 16)
                # per-block output stores (late phase; Pool queue is idle)
                for b in range(NBLK):
                    if b % 2 == 0:
                        gp.wait_ge(ocopA, b // 2 + 1)
                    else:
                        gp.wait_ge(ocopB, (b + 1) // 2)
                    gp.dma_start(
                        out=out[:, b * BW:(b + 1) * BW],
                        in_=o_sb[:, b * BW:(b + 1) * BW],
                    ).then_inc(odma, 16)
                gp.wait_ge(odma, 16 * NBLK)

            @block.vector
            def _(ve):
                ve.wait_ge(wsem, 16 * 11)
                # pad cols of tab_sb (beyond stripe coverage) stay zero
                ve.memset(tab_sb[:, NSTR * 128:EPAD], 0.0)
                with nc.allow_low_precision(reason="bf16 edge aggregation"):
                    for s in range(NSTR):
                        gi = int(grp_of[s])
                        ve.wait_ge([gsem0, gsem1, gsem2, gsem3][gi % 4],
                                   16 * (gi // 4 + 1))
                        if s >= RT:
                            ve.wait_ge(psem, s - (RT - 1))
                        s0 = groups[gi][0]
                        base = (gi % RGRP) * GW * 12 + int(o_s[s] - o_s[s0]) * 12
                        m = int(m_s[s])
                        ve.tensor_reduce(
                            out=t_sb[:, (s % RT) * 12:(s % RT + 1) * 12],
                            in_=g_sb[:, base:base + m * 12]
                                .rearrange("p (m j) -> p j m", j=12),
                            axis=mybir.AxisListType.X,
                            op=mybir.AluOpType.add,
                        ).then_inc(tsem, 1)
                        # lagged quad copy: after reduces of quad q+1, copy quad q
                        if s % 4 == 3 and s >= 7:
                            q = s // 4 - 1
                            lo, hi = quad_cols(q)
                            ve.wait_ge(psem, 4 * q + (hi - lo) // 128)
                            ve.tensor_copy(
                                out=tab_sb[:, lo:hi],
                                in_=ps_t[q % 2][:, 0:hi - lo],
                            ).then_inc(csem, 1)
                            if q >= 3 and (q - 3) % 2 == 0:
                                _ocopy_dve(ve, q - 3)
                for q in range(NQ - 2, NQ):
                    lo, hi = quad_cols(q)
                    ve.wait_ge(psem, 4 * q + (hi - lo) // 128)
                    ve.tensor_copy(
                        out=tab_sb[:, lo:hi],
                        in_=ps_t[q % 2][:, 0:hi - lo],
                    ).then_inc(csem, 1)
                for o in (20, 22, 24):
                    _ocopy_dve(ve, o)

            def _pe_block(te, b):
                te.wait_ge(csem, b + 1)
                te.wait_ge([xsem0, xsem1, xsem2, xsem3][b % 4],
                           16 * (b // 4 + 1))
                if b >= 4:
                    te.wait_ge(rsem, b - 3)
                for k in range(6):
                    te.matmul(
                        ps_z[b % 4][:],
                        w_sb[:, k * 128:(k + 1) * 128],
                        x_sb[:, (b % 4) * 6 * BW + k * BW:
                             (b % 4) * 6 * BW + (k + 1) * BW],
                        start=(k == 0),
                        stop=False,
                    )
                te.matmul(
                    ps_z[b % 4][:],
                    wcomb_sb[:],
                    tab_sb[:, b * BW:(b + 1) * BW],
                    start=False,
                    stop=True,
                ).then_inc(zsem, 1)
                te.wait_ge(rsem, b + 1)
                if b >= 2:
                    if b % 2 == 0:
                        te.wait_ge(ocopA, b // 2)
                    else:
                        te.wait_ge(ocopB, (b - 1) // 2)
                te.matmul(
                    ps_o[b % 2][:],
                    wc_sb[:],
                    zr_sb[:, (b % 2) * BW:(b % 2 + 1) * BW],
                    start=True,
                    stop=True,
                ).then_inc(osem, 1)

            @block.tensor
            def _(te):
                te.wait_ge(wsem, 16 * 11)
                for s in range(NSTR):
                    te.wait_ge(tsem, s + 1)
                    q = s // 4
                    if q >= 2:
                        te.wait_ge(csem, q - 1)
                    te.matmul(
                        ps_t[q % 2][:, (s % 4) * 128:(s % 4 + 1) * 128],
                        t_sb[:, (s % RT) * 12:(s % RT + 1) * 12],
                        ident_sb[:],
                        is_transpose=True,
                    ).then_inc(psem, 1)
                    # emit block b once its quad copy can complete (copy b is
                    # emitted on DVE after reduce of stripe 4b+7)
                    if s % 4 == 3 and s >= 11:
                        _pe_block(te, s // 4 - 2)
                for b in range(NQ - 3, NQ):
                    _pe_block(te, b)

    nc.compile()
    _prog_cache[key] = nc
    _prog_cache["nc"] = nc
    return nc


def _host_prep(inputs):
    f32 = np.float32
    x_email = np.asarray(inputs["x_email"], f32)
    x_url = np.asarray(inputs["x_url"], f32)
    x_sender = np.asarray(inputs["x_sender"], f32)

    # combined augmented gather table
    tab = np.zeros((NTAB, 12), f32)
    tab[:N_URL, 0:8] = x_url
    tab[:N_URL, 8] = 1.0
    tab[N_URL:N_URL + N_SENDER, 9] = x_sender[:, 0]
    tab[N_URL:N_URL + N_SENDER, 10] = 1.0

    # folded weights
    wroot = inputs["Wroot_ue"] + inputs["Wroot_se"]
    wer = np.ascontiguousarray((inputs["W_email"] @ wroot).astype(f32))
    wcomb = np.zeros((12, 128), f32)
    wcomb[0:8] = inputs["W_url"] @ inputs["Wrel_ue"]
    wcomb[8] = inputs["b_url"] @ inputs["Wrel_ue"]
    wcomb[9] = inputs["W_sender"][0] @ inputs["Wrel_se"]
    wcomb[10] = inputs["b_sender"] @ inputs["Wrel_se"]
    zbias = (inputs["brel_ue"] + inputs["brel_se"]
             + inputs["b_email"] @ wroot).astype(f32).reshape(128, 1)

    # ---- degree-sorted layout -------------------------------------------
    dst_all = np.concatenate([
        np.asarray(inputs["dst_ue"], np.int64),
        np.asarray(inputs["dst_se"], np.int64),
    ])
    srcrow_all = np.concatenate([
        np.asarray(inputs["src_ue"], np.int64),
        np.asarray(inputs["src_se"], np.int64) + N_URL,
    ]).astype(np.int32)
    E_TOT = dst_all.shape[0]

    deg = np.bincount(dst_all, minlength=N_EMAIL)
    perm = np.argsort(-deg, kind="stable")          # emails by degree desc
    rank = np.empty(N_EMAIL, np.int64)
    rank[perm] = np.arange(N_EMAIL)

    key = rank[dst_all]
    order = np.argsort(key, kind="stable")
    ks = key[order]
    ss = srcrow_all[order]
    starts = np.searchsorted(ks, np.arange(N_EMAIL))
    k_within = np.arange(E_TOT) - starts[ks]

    core = (ks % NCORE).astype(np.int64)
    pos = ks // NCORE
    stripe = pos // 128
    part = pos % 128

    deg_rank = deg[perm]                             # descending
    dr = np.zeros(NSTR * 128 * NCORE, np.int64)
    dr[:N_EMAIL] = deg_rank                          # rank-major: pos*8+core
    m_s = dr.reshape(NSTR, 128 * NCORE).max(axis=1)
    m_s = np.maximum(m_s, 1).astype(np.int64)
    o_s = np.zeros(NSTR + 1, np.int64)
    o_s[1:] = np.cumsum(m_s)
    NCOL = int(o_s[-1])

    # gather groups: consecutive stripes, <= GW columns each
    groups = []
    s = 0
    while s < NSTR:
        s1 = s + 1
        while s1 < NSTR and o_s[s1 + 1] - o_s[s] <= GW:
            s1 += 1
        assert o_s[s1] - o_s[s] <= GW, f"stripe {s} has m={m_s[s]} > GW"
        groups.append((s, int(s1)))
        s = int(s1)

    layout = {"m_s": m_s, "o_s": o_s, "groups": groups}
    _prog_cache["layout"] = layout

    # per-core src index arrays
    SRC = np.full((NCORE, 128, NCOL), ZROW, np.int32)
    SRC[core, part, o_s[stripe] + k_within] = ss

    identity = np.eye(128, dtype=f32).astype(BF)
    wer_bf = wer.astype(BF)
    wcomb_bf = wcomb.astype(BF)
    wc_bf = np.ascontiguousarray(np.asarray(inputs["Wc"], f32)).astype(BF)
    bc_np = np.asarray(inputs["bc"], f32).reshape(2, 1)
    tab_bf = tab.astype(BF)

    in_maps = []
    for c in range(NCORE):
        xTc = np.zeros((768, EPAD), f32)
        xTc[:, :EPC] = x_email[perm[c::NCORE]].T
        in_maps.append({
            "tab": tab_bf,
            "srcc": np.ascontiguousarray(SRC[c]),
            "xT": xTc.astype(BF),
            "wer": wer_bf,
            "wcomb": wcomb_bf,
            "wc": wc_bf,
            "bc": bc_np,
            "zbias": zbias,
            "ident": identity,
        })
    _prog_cache["perm"] = perm
    return in_maps


def kernel(**inputs):
    in_maps = _host_prep(inputs)
    nc = _build_program(_prog_cache["layout"])
    perm = _prog_cache["perm"]
    res = None
    last_exc = None
    for _attempt in range(3):
        try:
            res = run_bass_kernel_spmd(nc, in_maps, list(range(NCORE)))
            break
        except Exception as e:  # transient device wedge recovers on retry
            last_exc = e
            import time as _time
            _time.sleep(5.0)
    if res is None:
        raise last_exc
    out = np.empty((N_EMAIL, 2), np.float32)
    for c in range(NCORE):
        out[perm[c::NCORE]] = res.results[c]["out"][:, :EPC].T
    return out
